# revision 1
# baseline (speedup 1.0000x reference)
"""Trainium2 Bass kernel for nn_ContextualizationLayer.

Computation (per batch b):
  q = x @ Wq, k = x @ Wk  -> (L, K, HD)
  scores[h] = q_h @ k_h^T / sqrt(HD)          (L, L) per head
  alpha = softmax(scores, axis=-1)
  o = sum_h alpha[h] @ C_x[b, :, :, h]        (L, D)

Sharding: the 24 (batch, head) pairs are split 3-per-core across 8 cores
(cores 0-3: batch 0, cores 4-7: batch 1). Each core computes its 3 heads'
partial outputs; the host sums partials (the final reduction over heads).

Per-core kernel (SPMD, identical program on all 8 cores):
  - proj: one matmul per (head, chunk, i-group) computes q AND k together:
    the stationary operand packs [Wq_cols | Wk_cols] so the PSUM output has
    qT on partitions 0:64 and kT on partitions 64:128 -- half the matmul
    instructions of separate q/k projections.
  - per head, per i-span of 256 (pipelined one span ahead):
      scoresT[j, i] in 4 rounds of 4 j-chunks; each round's [128, 1024] PSUM
      tile is exp'd TWICE on ACT: once to fp8-e4m3 (E_hi) and once to fp16;
      DVE computes E_lo = e4m3(E16 - E_hi). A ones-column appended to C_x
      makes the AV matmul produce Z[i] in output column D (no partition
      reduction needed for the softmax denominator).
  - AV uses fp8 DoubleRow matmuls (2 contraction chunks of 128 j per
    instruction at 0.5 cycles/row) with error compensation:
      o ~= E_hi @ C8 + E_lo @ C8 + E_hi @ R8
    where C8 = e4m3(C_x), R8 = e4m3(C_x - C8). This runs 33% faster than a
    single fp16 AV pass while keeping rel err ~2e-3 (plain fp8 would be 4e-2).
  - epilogue: per-partition 1/Z multiply on DVE, fp16 partial out.

QK stays fp16 (softmax is too sensitive to fp8 score noise). Scores are
computed as exp(psum * ISCALE - SHIFT); the constant shift keeps E_hi under
e4m3's max (240) and cancels in the Z normalization.
"""

import os
import numpy as np
import ml_dtypes

B, L, D, K, HD = 2, 2048, 768, 12, 64
N_CORES = 8
HPC = 3            # heads per core
NCH = D // 128     # 6 contraction chunks for proj
NJ = L // 128      # 16 j-chunks
SPAN = 256         # i-span width
NSPAN = L // SPAN  # 8
ICS = SPAN // 128  # 2 i-chunks per span
NR = 4             # QK rounds per span (4 j-chunks each)
SHIFT = 3.5        # exp shift; cancels in Z normalization. Keeps E_hi =
                   # exp(score - SHIFT) under e4m3's max (240) for scaled
                   # scores up to ~9 (observed max ~7.9).
ISCALE = float(1.0 / np.sqrt(HD))

TRACE = False        # set True (or BASS_TRACE=1) to profile
LAST_RESULT = None   # BassKernelResults of the last run

_module_cache = {}


def _split_multiwait(nc, mybir):
    """The pinned walrus build supports at most one semaphore wait per
    instruction; Tile emits instructions carrying one wait per dependency
    semaphore. Hoist all-but-one wait onto single-wait nops immediately
    preceding the instruction on the same engine (semantically identical:
    semaphores are monotonic)."""
    counter = 0
    for f in nc.m.functions:
        for bb in f.blocks:
            out = []
            changed = False
            for inst in bb.instructions:
                si = inst.sync_info
                if si is not None and si.on_wait and len(si.on_wait) > 1:
                    waits = list(si.on_wait)
                    for w in waits[:-1]:
                        nop = mybir.InstNoOp(name=f"zz_waitsplit_{counter}", ins=[], outs=[])
                        counter += 1
                        nop.engine = inst.engine
                        nop.sync_info = type(si)(on_wait=[w], on_update=[])
                        out.append(nop)
                    inst.sync_info = type(si)(
                        on_wait=[waits[-1]], on_update=list(si.on_update)
                    )
                    changed = True
                out.append(inst)
            if changed:
                bb.instructions = out


def _build_module():
    import concourse.bass as bass
    import concourse.mybir as mybir
    import concourse.tile as tile
    from contextlib import ExitStack

    f32 = mybir.dt.float32
    f16 = mybir.dt.float16
    e4 = mybir.dt.float8e4
    EXP = mybir.ActivationFunctionType.Exp
    DR = mybir.MatmulPerfMode.DoubleRow

    nc = bass.Bass(
        "TRN2",
        target_bir_lowering=False,
        debug=False,
        enable_asserts=False,
        num_devices=N_CORES,
    )
    xT = nc.dram_tensor("xT", (D, L), f16, kind="ExternalInput").ap()
    # combined [Wq(64) | Wk(64)] columns per (head, chunk), SBUF layout
    wqk = nc.dram_tensor("wqk", (128, HPC * NCH * 128), f16, kind="ExternalInput").ap()
    # C8/R8: e4m3 main + residual, DoubleRow layout [h, jp, 128, 2, D+1]
    cx8 = nc.dram_tensor("cx8", (HPC, NJ // 2, 128, 2, D + 1), e4, kind="ExternalInput").ap()
    rx8 = nc.dram_tensor("rx8", (HPC, NJ // 2, 128, 2, D + 1), e4, kind="ExternalInput").ap()
    o3 = nc.dram_tensor("o3", (HPC, L, D), f16, kind="ExternalOutput").ap()

    with tile.TileContext(nc) as tc, ExitStack() as ctx:
        xt_pool = ctx.enter_context(tc.tile_pool(name="xt", bufs=24))
        w_pool = ctx.enter_context(tc.tile_pool(name="w", bufs=1))
        qk_pool = ctx.enter_context(tc.tile_pool(name="qk", bufs=6))
        c_pool = ctx.enter_context(tc.tile_pool(name="c8", bufs=18))
        r_pool = ctx.enter_context(tc.tile_pool(name="r8", bufs=18))
        e16_pool = ctx.enter_context(tc.tile_pool(name="e16", bufs=4))
        e8h_pool = ctx.enter_context(tc.tile_pool(name="e8h", bufs=10))
        e8l_pool = ctx.enter_context(tc.tile_pool(name="e8l", bufs=10))
        t_pool = ctx.enter_context(tc.tile_pool(name="t", bufs=8))
        z_pool = ctx.enter_context(tc.tile_pool(name="z", bufs=4))
        k_pool = ctx.enter_context(tc.tile_pool(name="konst", bufs=1))
        ps_qk = ctx.enter_context(tc.tile_pool(name="psqk", bufs=2, space="PSUM"))
        ps_av = ctx.enter_context(tc.tile_pool(name="psav", bufs=2, space="PSUM"))

        nshift = k_pool.tile([128, 1], f32, name="nshift", tag="konst")
        nc.gpsimd.memset(nshift[:], -SHIFT)

        wqk_sb = w_pool.tile([128, HPC * NCH * 128], f16, name="wqk_sb", tag="w")
        nc.sync.dma_start(wqk_sb[:], wqk[:])

        # x^T as 6x4 tiles of [128, 512], it-outer so head-0 proj can start
        # early; head-0's C8/R8 tiles are interleaved into the same stream.
        cxts = {0: ([None] * (NJ // 2), [None] * (NJ // 2))}

        def emit_cx(h, jp):
            ct = c_pool.tile([128, 2, D + 1], e4, name=f"c8_{h}_{jp}", tag="c8")
            rt = r_pool.tile([128, 2, D + 1], e4, name=f"r8_{h}_{jp}", tag="r8")
            nc.sync.dma_start(ct[:], cx8[h, jp])
            nc.sync.dma_start(rt[:], rx8[h, jp])
            cxts[h][0][jp] = ct
            cxts[h][1][jp] = rt

        xt = [[None] * (L // 512) for _ in range(NCH)]
        for it in range(L // 512):
            for c in range(NCH):
                t = xt_pool.tile([128, 512], f16, name=f"xt{c}_{it}", tag="xt")
                nc.sync.dma_start(
                    t[:], xT[c * 128 : (c + 1) * 128, it * 512 : (it + 1) * 512]
                )
                xt[c][it] = t
            emit_cx(0, 2 * it)
            emit_cx(0, 2 * it + 1)

        # ---- projections: one matmul per (h, c, it) computes q AND k:
        # lhsT = [Wq_cols(64) | Wk_cols(64)] -> psum partitions 0:64 = qT,
        # 64:128 = kT. qT is duplicated into rows 64:128 (SBUF->SBUF DMA
        # shifts partitions) so QK runs with both operands based at 64.
        qT, kT = [], []
        for h in range(HPC):
            qT.append(qk_pool.tile([128, L], f16, name=f"qT{h}", tag="qk"))
            kT.append(qk_pool.tile([128, L], f16, name=f"kT{h}", tag="qk"))

        def emit_proj(h):
            for it in range(L // 512):
                ps = ps_av.tile([128, 512], f32, name="ps_proj", tag="psav")
                for c in range(NCH):
                    nc.tensor.matmul(
                        ps[:, :],
                        wqk_sb[:, (h * NCH + c) * 128 : (h * NCH + c + 1) * 128],
                        xt[c][it][:],
                        start=(c == 0),
                        stop=(c == NCH - 1),
                    )
                sl = slice(it * 512, (it + 1) * 512)
                nc.vector.tensor_copy(qT[h][0:64, sl], ps[0:64, :])
                nc.vector.tensor_copy(kT[h][64:128, sl], ps[64:128, :])
                nc.sync.dma_start(qT[h][64:128, sl], qT[h][0:64, sl])

        # ---- per-span building blocks ----
        units = [(h, s) for h in range(HPC) for s in range(NSPAN)]
        banks = {}  # g -> list of NR (e8h, e8l) pairs

        def qk_round(hn, sn, r):
            """4 QK matmuls -> [128, 1024] psum -> exp to e4m3 (E_hi) and
            fp16 (E16) on ACT -> E_lo = E16 - E_hi on DVE."""
            isl = slice(sn * SPAN, (sn + 1) * SPAN)
            ps = ps_qk.tile([128, NR * SPAN], f32, name="ps_sc", tag="psqk")
            for jj in range(NR):
                j = NR * r + jj
                nc.tensor.matmul(
                    ps[:, jj * SPAN : (jj + 1) * SPAN],
                    kT[hn][64:128, j * 128 : (j + 1) * 128],
                    qT[hn][64:128, isl],
                    start=True,
                    stop=True,
                )
            e8h = e8h_pool.tile([128, NR, SPAN], e4, name="e8h", tag="e8h")
            e16 = e16_pool.tile([128, NR, SPAN], f16, name="e16", tag="e16")
            e8l = e8l_pool.tile([128, NR, SPAN], e4, name="e8l", tag="e8l")
            nc.scalar.activation(e8h[:], ps[:], EXP, scale=ISCALE, bias=nshift[:])
            nc.scalar.activation(e16[:], ps[:], EXP, scale=ISCALE, bias=nshift[:])
            nc.vector.tensor_sub(e8l[:], e16[:], e8h[:])
            return (e8h, e8l)

        def av_jp(ps_ic, ic, jp, ebanks, ctiles, rtiles, first, last):
            """One j-pair of the 3-pass compensated fp8 AV into ps_ic."""
            r, t = divmod(jp, 2)
            e8h, e8l = ebanks[r]
            csl = slice(ic * 128, (ic + 1) * 128)
            hi = e8h[:, 2 * t : 2 * t + 2, csl]
            lo = e8l[:, 2 * t : 2 * t + 2, csl]
            ct, rt = ctiles[jp], rtiles[jp]
            for pi, (lh, rh) in enumerate(((hi, ct), (lo, ct), (hi, rt))):
                st = first and pi == 0
                sp = last and pi == 2
                nc.tensor.matmul(ps_ic[:, 0:512], lh, rh[:, :, 0:512],
                                 start=st, stop=sp, perf_mode=DR)
                nc.tensor.matmul(ps_ic[:, 512 : D + 1], lh, rh[:, :, 512 : D + 1],
                                 start=st, stop=sp, perf_mode=DR)

        def emit_epilogue(h, gic, ps, nsplit=2):
            zinv = z_pool.tile([128, 1], f32, name="zinv", tag="z")
            nc.vector.reciprocal(zinv[:], ps[:, D : D + 1])
            tmp = t_pool.tile([128, D], f16, name="onorm", tag="t")
            step = D // nsplit
            for part in range(nsplit):
                dsl = slice(part * step, (part + 1) * step)
                nc.vector.tensor_scalar_mul(tmp[:, dsl], ps[:, dsl], zinv[:])
                nc.sync.dma_start(o3[h, gic * 128 : (gic + 1) * 128, dsl], tmp[:, dsl])

        # ---- emission schedule ----
        # proj h0, then QK(unit0) whose exps overlap proj h1/h2 on PE.
        emit_proj(0)
        h0, s0 = units[0]
        banks[0] = [qk_round(h0, s0, r) for r in range(NR)]
        emit_proj(1)
        emit_proj(2)

        for g, (h, s) in enumerate(units):
            ctiles, rtiles = cxts[h]
            ebanks = banks.pop(g)
            # prefetch next head's C8/R8 spread over spans 1..4
            if h + 1 < HPC and 1 <= s <= 4:
                if h + 1 not in cxts:
                    cxts[h + 1] = ([None] * (NJ // 2), [None] * (NJ // 2))
                emit_cx(h + 1, 2 * (s - 1))
                emit_cx(h + 1, 2 * (s - 1) + 1)

            last_unit = g + 1 >= len(units)
            if not last_unit:
                hn, sn = units[g + 1]
                nxt = []
                # weave next unit's QK rounds between this unit's AV groups
                # so the PE never waits on ACT exp throughput.
                nxt.append(qk_round(hn, sn, 0))
                nxt.append(qk_round(hn, sn, 1))
                ps0 = ps_av.tile([128, D + 1], f32, name="ps_av0", tag="psav")
                for jp in range(4):
                    av_jp(ps0, 0, jp, ebanks, ctiles, rtiles, jp == 0, False)
                nxt.append(qk_round(hn, sn, 2))
                for jp in range(4, NJ // 2):
                    av_jp(ps0, 0, jp, ebanks, ctiles, rtiles, False, jp == NJ // 2 - 1)
                emit_epilogue(h, s * ICS, ps0)
                nxt.append(qk_round(hn, sn, 3))
                ps1 = ps_av.tile([128, D + 1], f32, name="ps_av1", tag="psav")
                for jp in range(NJ // 2):
                    av_jp(ps1, 1, jp, ebanks, ctiles, rtiles, jp == 0, jp == NJ // 2 - 1)
                emit_epilogue(h, s * ICS + 1, ps1)
                banks[g + 1] = nxt
            else:
                for ic in range(ICS):
                    ps = ps_av.tile([128, D + 1], f32, name=f"ps_avl{ic}", tag="psav")
                    for jp in range(NJ // 2):
                        av_jp(ps, ic, jp, ebanks, ctiles, rtiles,
                              jp == 0, jp == NJ // 2 - 1)
                    emit_epilogue(h, s * ICS + ic, ps, nsplit=4)

    _split_multiwait(nc, mybir)
    return nc


def _get_module():
    if "nc" not in _module_cache:
        _module_cache["nc"] = _build_module()
    return _module_cache["nc"]


def kernel(x, C_x, Wq, Wk):
    global LAST_RESULT
    import concourse.bass_utils as bass_utils

    x = np.asarray(x, dtype=np.float32)
    C_x = np.asarray(C_x, dtype=np.float32)
    Wq = np.asarray(Wq, dtype=np.float32)
    Wk = np.asarray(Wk, dtype=np.float32)

    nc = _get_module()

    f16 = np.float16
    e4m3 = ml_dtypes.float8_e4m3
    xT_by_b = [np.ascontiguousarray(x[b].T).astype(f16) for b in range(B)]

    def w_layout(heads):
        # (D, K*HD) -> [128, HPC*NCH*128]: column block (h*NCH + c)*128
        # holds [Wq[c*128:(c+1)*128, head] | Wk[...]]
        arr = np.zeros((128, HPC * NCH * 128), dtype=f16)
        Wqh = Wq.reshape(D, K, HD)[:, heads, :]  # (D, HPC, HD)
        Wkh = Wk.reshape(D, K, HD)[:, heads, :]
        for h in range(HPC):
            for c in range(NCH):
                base = (h * NCH + c) * 128
                arr[:, base : base + 64] = Wqh[c * 128 : (c + 1) * 128, h, :]
                arr[:, base + 64 : base + 128] = Wkh[c * 128 : (c + 1) * 128, h, :]
        return arr

    in_maps = []
    for c in range(N_CORES):
        b = c // (N_CORES // B)
        g = c % (N_CORES // B)
        heads = [HPC * g + i for i in range(HPC)]
        c8 = np.zeros((HPC, NJ // 2, 128, 2, D + 1), dtype=e4m3)
        r8 = np.zeros((HPC, NJ // 2, 128, 2, D + 1), dtype=e4m3)
        for i, h in enumerate(heads):
            caug = np.ones((L, D + 1), dtype=np.float32)
            caug[:, :D] = C_x[b, :, :, h]
            c8f = caug.astype(e4m3)
            r8f = (caug - c8f.astype(np.float32)).astype(e4m3)
            c8[i] = c8f.reshape(NJ // 2, 2, 128, D + 1).transpose(0, 2, 1, 3)
            r8[i] = r8f.reshape(NJ // 2, 2, 128, D + 1).transpose(0, 2, 1, 3)
        in_maps.append(
            {
                "xT": xT_by_b[b],
                "wqk": w_layout(heads),
                "cx8": c8,
                "rx8": r8,
            }
        )

    trace = TRACE or bool(os.environ.get("BASS_TRACE"))
    if trace:
        try:
            from antenv.axon_hooks import get_axon_ntff_profile_hook  # noqa: F401
        except ImportError:
            trace = False  # no NTFF hook in this axon client install

    # The first execution after a fresh NEFF load occasionally hits a
    # transient NRT_EXEC_UNIT_UNRECOVERABLE on this fabric; retry.
    last_exc = None
    for attempt in range(3):
        try:
            res = bass_utils.run_bass_kernel_spmd(
                nc,
                in_maps,
                core_ids=list(range(N_CORES)),
                trace=trace,
            )
            break
        except Exception as e:  # noqa: BLE001
            last_exc = e
            import time

            time.sleep(2.0)
    else:
        raise last_exc
    LAST_RESULT = res

    out = np.zeros((B, L, D), dtype=np.float32)
    for c in range(N_CORES):
        b = c // (N_CORES // B)
        out[b] += res.results[c]["o3"].astype(np.float32).sum(axis=0)
    return out

